# revision 1
# baseline (speedup 1.0000x reference)
"""DMVFlow per-state diagonal-Gaussian log-density kernel for 8 TRN2 NeuronCores.

density[b,t,k] = log_norm - 0.5*(s2[b,t] - 2*cross[b,t,k] + m2[k])
  with  log_norm = -0.5*(D*log(2pi) + sum_d log var[d])
        s2[b,t]  = sum_d s[b,t,d]^2 / var[d]
        cross    = sum_d s[b,t,d] * means[k,d] / var[d]
        m2[k]    = sum_d means[k,d]^2 / var[d]

Sharding: data-parallel over batch (32 sentences per core), means/var replicated.

Device program (per core, rows = 32*256 = 8192 token positions):
  - s arrives pre-transposed on host as st[c, p, n] = s[n, 128*c + p] in fp16
    (contraction dim d = 128*c + p on SBUF partitions; n = token row).
  - PSUM[k, n] accumulates 12 matmuls per 512-row tile:
      6 x (W chunk)     : W[d,k] = means[k,d]/var[d]      -> cross term
      6 x (V chunk)     : V[d,k] = -0.5/var[d] (repl. k)  -> -0.5*s2 term,
                          moving operand = s^2 (squared on ACT/DVE engines)
  - out[k, n] = PSUM + c[k],  c[k] = log_norm - 0.5*m2[k]  (DVE tensor_scalar)
  - Host transposes the (128, 8192) per-core result back to (32, 256, 128).
"""

import numpy as np

N_CORES = 8
B, T, D, K = 256, 256, 768, 128
BPC = B // N_CORES          # batches per core
R = BPC * T                 # rows (token positions) per core = 8192
TN = 512                    # rows per tile
NT = R // TN                # tiles per core = 16
C = D // 128                # contraction chunks = 6

_NC = None                  # cached bass program (build once per process)
OUT_F16 = True              # device writes fp16 output (host upcasts); ~4e-4 scale-rel err


def _build_nc():
    from contextlib import ExitStack

    import concourse.bacc as bacc
    import concourse.tile as tile
    from concourse import mybir

    f16 = mybir.dt.float16
    f32 = mybir.dt.float32

    nc = bacc.Bacc(None, target_bir_lowering=False, debug=False)

    st = nc.dram_tensor("st", [C, 128, R], f16, kind="ExternalInput")
    wt = nc.dram_tensor("wt", [128, C, K], f16, kind="ExternalInput")
    vt = nc.dram_tensor("vt", [128, C, K], f16, kind="ExternalInput")
    cv = nc.dram_tensor("cv", [K, 1], f32, kind="ExternalInput")
    out = nc.dram_tensor("out", [K, R], f32, kind="ExternalOutput")

    with tile.TileContext(nc) as tc, ExitStack() as ctx:
        consts = ctx.enter_context(tc.tile_pool(name="consts", bufs=1))
        inp = ctx.enter_context(tc.tile_pool(name="inp", bufs=10))
        sq = ctx.enter_context(tc.tile_pool(name="sq", bufs=6))
        outp = ctx.enter_context(tc.tile_pool(name="outp", bufs=6))
        psum = ctx.enter_context(
            tc.tile_pool(name="psum", bufs=8, space="PSUM")
        )

        # consts ride the scalar ring so tile 0's chunks stream unblocked on sync
        w_sb = consts.tile([128, C, K], f16)
        nc.scalar.dma_start(w_sb[:], wt[:])
        v_sb = consts.tile([128, C, K], f16)
        nc.scalar.dma_start(v_sb[:], vt[:])
        c_sb = consts.tile([K, 1], f32)
        nc.scalar.dma_start(c_sb[:], cv[:])

        for t in range(NT):
            n0 = t * TN
            s_t = inp.tile([128, C, TN], f16, tag="s")
            if t == 0:
                # chunk-granular first load: matmul c starts once chunk c lands
                for c in range(C):
                    nc.sync.dma_start(s_t[:, c, :], st[c, :, n0 : n0 + TN])
            else:
                nc.sync.dma_start(
                    s_t[:], st[:, :, n0 : n0 + TN].rearrange("c p n -> p c n")
                )

            # squares: ACT takes 3 chunks, DVE takes 3 (split keeps both busy
            # under PE's 12-matmul tile time)
            sq_a = sq.tile([128, 3, TN], f16, tag="sqa")
            nc.scalar.activation(
                sq_a[:], s_t[:, 0:3, :], mybir.ActivationFunctionType.Square
            )
            sq_d = sq.tile([128, 3, TN], f16, tag="sqd")
            nc.vector.tensor_mul(sq_d[:], s_t[:, 3:6, :], s_t[:, 3:6, :])

            acc = psum.tile([K, TN], f32)
            for c in range(C):
                nc.tensor.matmul(
                    acc[:], w_sb[:, c, :], s_t[:, c, :],
                    start=(c == 0), stop=False,
                )
            for c in range(3):
                nc.tensor.matmul(
                    acc[:], v_sb[:, c, :], sq_a[:, c, :],
                    start=False, stop=False,
                )
            for c in range(3):
                nc.tensor.matmul(
                    acc[:], v_sb[:, 3 + c, :], sq_d[:, c, :],
                    start=False, stop=(c == 2),
                )

            o_t = outp.tile([K, TN], f32, tag="o")
            nc.vector.tensor_scalar_add(o_t[:], acc[:], c_sb[:])
            # scalar-engine HWDGE ring: keeps stores off the input DMA ring
            nc.scalar.dma_start(out[:, n0 : n0 + TN], o_t[:])

    return nc


def _build_nc_raw():
    """Hand-scheduled variant: no TileContext, manual semaphores.

    Engine roles:
      sync   - input DMAs (s tiles; chunk-split for tile 0)
      scalar - const DMAs, ACT squares (chunks 0-2), output DMAs
      vector - DVE squares (chunks 3-5), psum+c[k] combine
      tensor - 12 matmuls per tile
    """
    import concourse.bacc as bacc
    from concourse import mybir

    f16 = mybir.dt.float16
    f32 = mybir.dt.float32

    NIN = 12     # input s-tile slots
    NSQ = 6      # square-tile slots
    NPS = 8      # psum banks
    NOUT = 6     # output-tile slots

    nc = bacc.Bacc(None, target_bir_lowering=False, debug=False)

    st = nc.dram_tensor("st", [C, 128, R], f16, kind="ExternalInput")
    wv = nc.dram_tensor("wv", [128, 2 * C, K], f16, kind="ExternalInput")
    cv = nc.dram_tensor("cv", [K, 1], f32, kind="ExternalInput")
    fout = f16 if OUT_F16 else f32
    out = nc.dram_tensor("out", [K, R], fout, kind="ExternalOutput")

    from contextlib import ExitStack

    with ExitStack() as ctx:
        e = ctx.enter_context
        s_sb = e(nc.sbuf_tensor([128, NIN, C, TN], f16))
        sqa_sb = e(nc.sbuf_tensor([128, NSQ, 3, TN], f16))
        sqd_sb = e(nc.sbuf_tensor([128, NSQ, 3, TN], f16))
        o_sb = e(nc.sbuf_tensor([K, NOUT, TN], fout))
        wv_sb = e(nc.sbuf_tensor([128, 2 * C, K], f16))
        c_sb = e(nc.sbuf_tensor([K, 1], f32))
        warm_sb = e(nc.sbuf_tensor([128, 128], f16))
        w_sb = wv_sb[:, 0:C, :]
        v_sb = wv_sb[:, C : 2 * C, :]
        ps = [
            e(nc.psum_tensor(f"ps{i}", [K, TN], f32)) for i in range(NPS)
        ]

        # DMA completion sems rotate per buffer slot: DMAs on one ring can
        # complete out of order across the 16 SDMA engines, so a shared
        # counter only has race-free wait points at all-complete boundaries.
        in_semsA = [e(nc.semaphore(f"inA{j}")) for j in range(NIN)]
        in_semsB = [e(nc.semaphore(f"inB{j}")) for j in range(NIN)]
        out_sems = [e(nc.semaphore(f"os{j}")) for j in range(NOUT)]
        wv_sem = e(nc.semaphore("wv_sem"))      # +16 when weights resident
        cv_sem = e(nc.semaphore("cv_sem"))      # +16 when c[k] resident
        warm_sem = e(nc.semaphore("warm_sem"))  # scratch zeroed for PE warmup
        sqa_sem = e(nc.semaphore("sqa_sem"))    # +1 per ACT square tile
        sqd_sem = e(nc.semaphore("sqd_sem"))    # +1 per DVE square tile
        pe_sem = e(nc.semaphore("pe_sem"))      # +1 per finished MM group
        dve_sem = e(nc.semaphore("dve_sem"))    # +1 per combine
        blk = e(nc.Block())

        def in_wait(t, half):
            # (sem, value): "half h of s tile t resident"
            sems = in_semsA if half == 0 else in_semsB
            return sems[t % NIN], 16 * (t // NIN + 1)

        def wait_half(eng, t, half):
            sem, v = in_wait(t, half)
            eng.wait_ge(sem, v)

        @blk.sync
        def _(eng):
            eng.dma_start(wv_sb[:], wv[:]).then_inc(wv_sem, 16)
            for t in range(NT):
                n0 = t * TN
                slot = t % NIN
                if t >= NIN:
                    eng.wait_ge(pe_sem, t - NIN + 1)
                if t != 1:  # tile 1's half A is issued on the scalar ring
                    eng.dma_start(
                        s_sb[:, slot, 0:3, :],
                        st[0:3, :, n0 : n0 + TN].rearrange("c p n -> p c n"),
                    ).then_inc(in_semsA[slot], 16)
                eng.dma_start(
                    s_sb[:, slot, 3:6, :],
                    st[3:6, :, n0 : n0 + TN].rearrange("c p n -> p c n"),
                ).then_inc(in_semsB[slot], 16)

        L_C = 1   # combine lags squares by 1 tile (keeps DVE squares ahead of PE)
        L_S = 3   # store lags squares by 3 tiles (keeps ACT squares ahead of PE)

        def emit_store(eng, t):
            eng.wait_ge(dve_sem, t + 1)
            eng.dma_start(
                out[:, t * TN : (t + 1) * TN], o_sb[:, t % NOUT, :]
            ).then_inc(out_sems[t % NOUT], 16)

        @blk.gpsimd
        def _(eng):
            eng.memset(warm_sb[:], 0.0).then_inc(warm_sem, 1)
            eng.dma_start(c_sb[:], cv[:]).then_inc(cv_sem, 16)

        @blk.scalar
        def _(eng):
            eng.dma_start(
                s_sb[:, 1, 0:3, :],
                st[0:3, :, TN : 2 * TN].rearrange("c p n -> p c n"),
            ).then_inc(in_semsA[1], 16)
            for t in range(NT):
                slot = t % NIN
                qslot = t % NSQ
                if t >= NSQ:
                    eng.wait_ge(pe_sem, t - NSQ + 1)
                wait_half(eng, t, 0)
                nc.scalar.activation(
                    sqa_sb[:, qslot, :, :],
                    s_sb[:, slot, 0:3, :],
                    mybir.ActivationFunctionType.Square,
                ).then_inc(sqa_sem, 1)
                if t >= L_S:
                    emit_store(eng, t - L_S)
            for t in range(NT - L_S, NT):
                emit_store(eng, t)
            for j in range(NOUT):
                eng.wait_ge(out_sems[j], 16 * (NT // NOUT))

        @blk.vector
        def _(eng):
            def emit_combine(tc_):
                if tc_ == 0:
                    eng.wait_ge(cv_sem, 16)
                eng.wait_ge(pe_sem, tc_ + 1)
                if tc_ >= NOUT:
                    # slot's previous occupant (tile tc_-NOUT) must be stored
                    eng.wait_ge(out_sems[tc_ % NOUT], 16 * (tc_ // NOUT))
                nc.vector.tensor_scalar_add(
                    o_sb[:, tc_ % NOUT, :], ps[tc_ % NPS][:], c_sb[:]
                ).then_inc(dve_sem, 1)

            for t in range(NT):
                slot = t % NIN
                qslot = t % NSQ
                if t >= NSQ:
                    eng.wait_ge(pe_sem, t - NSQ + 1)
                wait_half(eng, t, 1)
                nc.vector.tensor_mul(
                    sqd_sb[:, qslot, :, :],
                    s_sb[:, slot, 3:6, :],
                    s_sb[:, slot, 3:6, :],
                ).then_inc(sqd_sem, 1)
                if t >= L_C:
                    emit_combine(t - L_C)
            for tc_ in range(NT - L_C, NT):
                emit_combine(tc_)

        @blk.tensor
        def _(eng):
            # HAM warmup: ~3.5us of throwaway matmuls on zeroed scratch while
            # the first input tile streams in, so real matmuls start at 2.4GHz
            eng.wait_ge(warm_sem, 1)
            NWARM = 100
            for j in range(NWARM):
                nc.tensor.matmul(
                    ps[NPS - 1][:, 0:128], warm_sb[:], warm_sb[:],
                    start=(j == 0), stop=(j == NWARM - 1),
                )
            eng.wait_ge(wv_sem, 16)  # weights resident
            for t in range(NT):
                slot = t % NIN
                qslot = t % NSQ
                acc = ps[t % NPS]
                if t >= NPS:
                    eng.wait_ge(dve_sem, t - NPS + 1)
                for c in range(C):
                    mm = nc.tensor.matmul(
                        acc[:], w_sb[:, c, :], s_sb[:, slot, c, :],
                        start=(c == 0), stop=False,
                    )
                    if c in (0, 3):
                        sem, v = in_wait(t, 0 if c == 0 else 1)
                        mm._wait_ge(sem, v)
                for c in range(3):
                    mm = nc.tensor.matmul(
                        acc[:], v_sb[:, c, :], sqa_sb[:, qslot, c, :],
                        start=False, stop=False,
                    )
                    if c == 0:
                        mm._wait_ge(sqa_sem, t + 1)
                for c in range(3):
                    mm = nc.tensor.matmul(
                        acc[:], v_sb[:, 3 + c, :], sqd_sb[:, qslot, c, :],
                        start=False, stop=(c == 2),
                    )
                    if c == 0:
                        mm._wait_ge(sqd_sem, t + 1)
                mm.then_inc(pe_sem, 1)

    return nc


_RAW = True


def _scrub_debug_paths(nc):
    """Normalize per-instruction debug info (absolute file paths, tracebacks)
    so the serialized BIR is byte-identical regardless of where this file
    lives -- keeps the neuronxcc compile cache warm across directories."""
    import dataclasses

    def fix(obj):
        for attr in ("debug", "ant_debug"):
            dbg = getattr(obj, attr, None)
            if dbg is not None and getattr(dbg, "filename", None):
                setattr(
                    obj,
                    attr,
                    dataclasses.replace(
                        dbg, filename="kernel.py", ant_traceback=None
                    ),
                )

    for bb in nc.main_func.blocks:
        for ins in bb.instructions:
            fix(ins)
    for fn in nc.m.functions:
        for alloc in fn.allocations:
            fix(alloc)
            for ml in getattr(alloc, "memorylocations", None) or []:
                fix(ml)


def _get_nc():
    global _NC
    if _NC is None:
        import concourse.bass as bass

        _NC = _build_nc_raw() if _RAW else _build_nc()
        _NC.compile()            # Bacc passes (reg alloc, sem gen, ...)
        _scrub_debug_paths(_NC)  # after compile so pass-inserted insts are hit
        bass.Bass.finalize(_NC)  # freeze (Bacc.finalize would re-run compile)
    return _NC


def prep_in_maps(s, means, var):
    s = np.asarray(s)
    means = np.asarray(means, dtype=np.float64)
    var = np.asarray(var, dtype=np.float64)

    inv = 1.0 / var
    w = np.ascontiguousarray(
        (means * inv[None, :]).T.reshape(C, 128, K).transpose(1, 0, 2)
    ).astype(np.float16)                                   # [p, c, k]
    v = np.ascontiguousarray(
        np.broadcast_to(
            (-0.5 * inv).reshape(C, 128, 1).transpose(1, 0, 2), (128, C, K)
        )
    ).astype(np.float16)                                   # [p, c, k]
    log_norm = -0.5 * (D * np.log(2.0 * np.pi) + np.sum(np.log(var)))
    m2 = (means * means) @ inv                             # (K,)
    c_full = log_norm - 0.5 * m2                           # (K,)
    # center the device output near zero so an fp16 store only rounds the
    # +-~300 dynamic part; the per-state offset h[k] is re-added on host
    m_s2 = -0.5 * np.sum(inv)                              # E[-0.5*s2], s~N(0,1)
    cvec = np.full((K, 1), -m_s2, dtype=np.float32)
    hvec = (c_full + m_s2).astype(np.float32)              # (K,)

    wv = np.ascontiguousarray(np.concatenate([w, v], axis=1))  # [p, 2C, k]

    s16 = s.astype(np.float16).reshape(N_CORES, R, D)
    in_maps = []
    for i in range(N_CORES):
        st_i = np.ascontiguousarray(s16[i].T).reshape(C, 128, R)
        # superset of tensor names; each builder picks what it declares
        in_maps.append(
            {"st": st_i, "wt": w, "vt": v, "wv": wv, "cv": cvec}
        )
    return in_maps, hvec


def run_device(in_maps, trace=False, trace_kwargs=None):
    from concourse.bass_utils import run_bass_kernel_spmd

    return run_bass_kernel_spmd(
        _get_nc(),
        in_maps,
        list(range(N_CORES)),
        trace=trace,
        **(trace_kwargs or {}),
    )


def assemble(results, hvec):
    full = np.empty((B, T, K), dtype=np.float32)
    for i in range(N_CORES):
        o = np.asarray(results[i]["out"])                  # (K, R)
        full[i * BPC : (i + 1) * BPC] = (
            o.T.reshape(BPC, T, K).astype(np.float32) + hvec[None, None, :]
        )
    return full


def kernel(s, means, var):
    in_maps, hvec = prep_in_maps(s, means, var)
    br = run_device(in_maps)
    return assemble(br.results, hvec)



# revision 2
# speedup vs baseline: 1.5619x; 1.5619x over previous
"""DMVFlow per-state diagonal-Gaussian log-density kernel for 8 TRN2 NeuronCores.

density[b,t,k] = log_norm - 0.5*(s2[b,t] - 2*cross[b,t,k] + m2[k])
  with  log_norm = -0.5*(D*log(2pi) + sum_d log var[d])
        s2[b,t]  = sum_d s[b,t,d]^2 / var[d]
        cross    = sum_d s[b,t,d] * means[k,d] / var[d]
        m2[k]    = sum_d means[k,d]^2 / var[d]

Only cross[b,t,k] couples (b,t) with k; the per-row term (log_norm - 0.5*s2)
and per-state term (-0.5*m2) are rank-1 in the output and are computed exactly
on the host and added during assembly.  The device therefore runs a single
fp8(e4m3) GEMM per core: cross = s @ (means/var).T, using DoubleRow perf mode
(two 128-deep k-tiles per instruction).

Sharding: data-parallel over batch (32 sentences per core), weights replicated.

Device program (per core, rows = 32*256 = 8192 token positions):
  - s arrives as st[p, t, c, n] = fp8(s[row = t*512 + n, d = c*128 + p]):
    contraction dim on SBUF partitions, one contiguous 3KB line per
    partition per 512-row tile (large DMA packets; DMA engines are the
    bottleneck at ~27 GB/s/engine x 16 engines).
  - PSUM[k, n] accumulates 3 DoubleRow matmuls per tile (256-deep each).
  - DVE casts PSUM fp32 -> fp16 SBUF; scalar ring stores to DRAM.
  - Host adds rowvec[b,t] + colvec[k] in fp32 during assembly.
"""

import numpy as np

N_CORES = 8
B, T, D, K = 256, 256, 768, 128
BPC = B // N_CORES          # batches per core
R = BPC * T                 # rows (token positions) per core = 8192
TN = 512                    # rows per tile (one PSUM bank)
NT = R // TN                # tiles per core = 16
C = D // 128                # contraction chunks = 6
G = C // 2                  # DoubleRow double-chunks = 3

_NC = None                  # cached bass program (build once per process)


def _build_nc_fp8():
    """Hand-scheduled fp8 DoubleRow kernel: no TileContext, manual semaphores.

    Engine roles:
      sync   - 16 input-tile DMAs, no waits (16 dedicated SBUF slots)
      scalar - weight DMA, output stores
      vector - PSUM -> fp16 SBUF cast
      tensor - warmup + 3 DoubleRow matmuls per tile
      gpsimd - zeroes the PE-warmup scratch
    """
    from contextlib import ExitStack

    import concourse.bacc as bacc
    from concourse import mybir

    f8 = mybir.dt.float8e4
    f16 = mybir.dt.float16
    f32 = mybir.dt.float32
    DR = mybir.MatmulPerfMode.DoubleRow

    NPS = 8      # psum banks
    NOUT = 6     # output-tile slots
    NWARM = 80   # PE clock-ramp matmuls on zeroed scratch

    nc = bacc.Bacc(None, target_bir_lowering=False, debug=False)

    st = nc.dram_tensor("st", [128, NT, C, TN], f8, kind="ExternalInput")
    wv = nc.dram_tensor("wv", [128, C, K], f8, kind="ExternalInput")
    out = nc.dram_tensor("out", [K, R], f16, kind="ExternalOutput")

    with ExitStack() as ctx:
        e = ctx.enter_context
        s_sb = e(nc.sbuf_tensor([128, NT, C, TN], f8))
        o_sb = e(nc.sbuf_tensor([K, NOUT, TN], f16))
        wv_sb = e(nc.sbuf_tensor([128, C, K], f8))
        warm_sb = e(nc.sbuf_tensor([128, 128], f16))
        ps = [e(nc.psum_tensor(f"ps{i}", [K, TN], f32)) for i in range(NPS)]

        in_sems = [e(nc.semaphore(f"in{t}")) for t in range(NT)]
        out_sems = [e(nc.semaphore(f"os{j}")) for j in range(NOUT)]
        wv_sem = e(nc.semaphore("wv_sem"))      # +16 when weights resident
        warm_sem = e(nc.semaphore("warm_sem"))  # scratch zeroed for PE warmup
        pe_sem = e(nc.semaphore("pe_sem"))      # +1 per finished MM group
        dve_sem = e(nc.semaphore("dve_sem"))    # +1 per cast
        blk = e(nc.Block())

        @blk.sync
        def _(eng):
            for t in range(NT):
                eng.dma_start(s_sb[:, t, :, :], st[:, t, :, :]).then_inc(
                    in_sems[t], 16
                )

        @blk.gpsimd
        def _(eng):
            eng.memset(warm_sb[:], 0.0).then_inc(warm_sem, 1)

        @blk.scalar
        def _(eng):
            eng.dma_start(wv_sb[:], wv[:]).then_inc(wv_sem, 16)
            for t in range(NT):
                eng.wait_ge(dve_sem, t + 1)
                eng.dma_start(
                    out[:, t * TN : (t + 1) * TN], o_sb[:, t % NOUT, :]
                ).then_inc(out_sems[t % NOUT], 16)
            for j in range(NOUT):
                uses = len(range(j, NT, NOUT))
                eng.wait_ge(out_sems[j], 16 * uses)

        @blk.vector
        def _(eng):
            for t in range(NT):
                eng.wait_ge(pe_sem, t + 1)
                if t >= NOUT:
                    # slot's previous occupant (tile t-NOUT) must be stored
                    eng.wait_ge(out_sems[t % NOUT], 16 * (t // NOUT))
                nc.vector.tensor_scalar_mul(
                    o_sb[:, t % NOUT, :], ps[t % NPS][:], 1.0
                ).then_inc(dve_sem, 1)

        @blk.tensor
        def _(eng):
            # HAM warmup: throwaway matmuls on zeroed scratch while the first
            # input tile streams in, so real matmuls start at 2.4GHz
            eng.wait_ge(warm_sem, 1)
            for j in range(NWARM):
                nc.tensor.matmul(
                    ps[NPS - 1][:, 0:128], warm_sb[:], warm_sb[:],
                    start=(j == 0), stop=(j == NWARM - 1),
                )
            eng.wait_ge(wv_sem, 16)  # weights resident
            for t in range(NT):
                acc = ps[t % NPS]
                if t >= NPS:
                    eng.wait_ge(dve_sem, t - NPS + 1)
                for g in range(G):
                    mm = nc.tensor.matmul(
                        acc[:],
                        wv_sb[:, 2 * g : 2 * g + 2, :],
                        s_sb[:, t, 2 * g : 2 * g + 2, :],
                        start=(g == 0), stop=(g == G - 1),
                        perf_mode=DR,
                    )
                    if g == 0:
                        mm._wait_ge(in_sems[t], 16)
                mm.then_inc(pe_sem, 1)

    return nc


def _scrub_debug_paths(nc):
    """Normalize per-instruction debug info (absolute file paths, tracebacks)
    so the serialized BIR is byte-identical regardless of where this file
    lives -- keeps the neuronxcc compile cache warm across directories."""
    import dataclasses

    def fix(obj):
        for attr in ("debug", "ant_debug"):
            dbg = getattr(obj, attr, None)
            if dbg is not None and getattr(dbg, "filename", None):
                setattr(
                    obj,
                    attr,
                    dataclasses.replace(
                        dbg, filename="kernel.py", ant_traceback=None
                    ),
                )

    for bb in nc.main_func.blocks:
        for ins in bb.instructions:
            fix(ins)
    for fn in nc.m.functions:
        for alloc in fn.allocations:
            fix(alloc)
            for ml in getattr(alloc, "memorylocations", None) or []:
                fix(ml)


def _get_nc():
    global _NC
    if _NC is None:
        import concourse.bass as bass

        _NC = _build_nc_fp8()
        _NC.compile()            # Bacc passes (reg alloc, sem gen, ...)
        _scrub_debug_paths(_NC)  # after compile so pass-inserted insts are hit
        bass.Bass.finalize(_NC)  # freeze (Bacc.finalize would re-run compile)
    return _NC


def prep_in_maps(s, means, var):
    import ml_dtypes

    f8np = ml_dtypes.float8_e4m3

    s = np.asarray(s, dtype=np.float32)
    means64 = np.asarray(means, dtype=np.float64)
    var64 = np.asarray(var, dtype=np.float64)

    inv = 1.0 / var64
    # W[d, k] = means[k, d] / var[d], packed as wv[p, c, k] with d = c*128 + p
    W = (means64 * inv[None, :]).T                          # (D, K)
    wv8 = np.ascontiguousarray(
        W.astype(np.float32).reshape(C, 128, K).transpose(1, 0, 2)
    ).astype(f8np)                                          # [p, c, k]

    # exact rank-1 terms, added on host during assembly
    log_norm = -0.5 * (D * np.log(2.0 * np.pi) + np.sum(np.log(var64)))
    m2 = (means64 * means64) @ inv                          # (K,)
    colvec = (-0.5 * m2).astype(np.float64)                 # (K,)
    s2 = np.einsum(
        "rd,d->r", (s.astype(np.float64) ** 2).reshape(-1, D), inv
    )                                                       # (B*T,)
    rowvec = (log_norm - 0.5 * s2).reshape(B, T)            # (B, T) fp64

    s8 = s.astype(f8np).reshape(N_CORES, NT, TN, C, 128)    # [i, t, n, c, p]
    in_maps = []
    for i in range(N_CORES):
        st_i = np.ascontiguousarray(s8[i].transpose(3, 0, 2, 1))  # [p,t,c,n]
        in_maps.append({"st": st_i, "wv": wv8})
    return in_maps, (rowvec, colvec)


def run_device(in_maps, trace=False, trace_kwargs=None):
    from concourse.bass_utils import run_bass_kernel_spmd

    return run_bass_kernel_spmd(
        _get_nc(),
        in_maps,
        list(range(N_CORES)),
        trace=trace,
        **(trace_kwargs or {}),
    )


def assemble(results, aux):
    rowvec, colvec = aux
    add = rowvec[:, :, None] + colvec[None, None, :]        # (B, T, K) fp64
    full = np.empty((B, T, K), dtype=np.float32)
    for i in range(N_CORES):
        o = np.asarray(results[i]["out"])                   # (K, R) fp16
        full[i * BPC : (i + 1) * BPC] = (
            o.T.reshape(BPC, T, K).astype(np.float64)
            + add[i * BPC : (i + 1) * BPC]
        ).astype(np.float32)
    return full


def kernel(s, means, var):
    in_maps, aux = prep_in_maps(s, means, var)
    br = run_device(in_maps)
    return assemble(br.results, aux)
